# revision 3
# baseline (speedup 1.0000x reference)
"""KNN classifier (N_TRAIN=65536, N_TEST=4096, DIM=512, k=5, 10 classes)
on 8 Trainium2 NeuronCores.

Strategy (reference-set parallel, class-bucketed):
  - Host reorders X_train by class and deals each class round-robin-contiguous
    across the 8 cores into fixed-size buckets of B=864 slots (padded).
  - Each core computes scores[t, n] = X_test[t]·x_n - 0.5*||x_n||^2 for its
    8640 bucket slots (monotone in -distance; the per-test ||t||^2 term and
    the sqrt are rank-irrelevant).  Exact-fp32-level precision via fp16 hi/lo
    splitting: cross = hi_t*hi_x + lo_t*hi_x + hi_t*lo_x (products of fp16
    pairs are exact in fp32 accumulation).  The -0.5||x||^2 - padding term
    rides as a K=2 matmul of ones against an fp16 hi/lo pair.
  - Per test row and per class bucket, DVE Max8 returns the 8 best scores.
    No indices needed: the class is the bucket.  Output [4096, 10*8] f32.
  - Host merges 8 cores x 10 classes x top-8 -> global top-5 -> mode with
    torch.mode tie semantics (smallest label wins).
"""

import functools
import os
import sys

sys.path.insert(0, "/opt/trn_rl_repo")

import numpy as np

NCORES = 8
P = 128
DIM = 512
KT = DIM // P  # 4
NTRAIN = 65536
NTEST = 4096
NCLASSES = 10
NNEIGH = 5
B = 864  # per-(core, class) bucket size
NTOT = NCLASSES * B  # 8640
NTILES = (NTOT + 511) // 512  # 17
AUGF = 512 * ((NTILES + 3) // 4)  # 2560
MT = NTEST // P  # 32 test tiles
PAD_SCORE = -60000.0  # fp16-representable, far below any real score

LAST_EXEC_TIME_NS = None  # set when KNN_TRACE=1


@functools.cache
def _build():
    from concourse import bacc
    import concourse.mybir as mybir
    import concourse.tile as tile

    fp16 = mybir.dt.float16
    f32 = mybir.dt.float32

    nc = bacc.Bacc(trn_type="TRN2")
    # test side, hi rows 0..511 then lo rows 512..1023
    xtT = nc.dram_tensor("xtT", [2 * DIM, NTEST], fp16, kind="ExternalInput")
    # train side (per-core bucketed shard), hi rows then lo rows
    xnT = nc.dram_tensor("xnT", [2 * DIM, NTOT], fp16, kind="ExternalInput")
    # packed aug rows: n-tile n -> partitions 32*(n%4) (hi), +1 (lo),
    # free offset 512*(n//4)
    xaug = nc.dram_tensor("xaug", [P, AUGF], fp16, kind="ExternalInput")
    ones = nc.dram_tensor("ones", [P, P], fp16, kind="ExternalInput")
    topv = nc.dram_tensor("topv", [NTEST, NCLASSES * 8], f32, kind="ExternalOutput")

    with tile.TileContext(nc) as tc:
        with (
            tc.tile_pool(name="xn", bufs=1) as xn_pool,
            tc.tile_pool(name="xt", bufs=3) as xt_pool,
            tc.tile_pool(name="score", bufs=1) as score_pool,
            tc.tile_pool(name="aug", bufs=1) as aug_pool,
            tc.tile_pool(name="ones", bufs=1) as ones_pool,
            tc.tile_pool(name="outp", bufs=3) as out_pool,
            tc.tile_pool(name="psum", bufs=8, space="PSUM") as psum_pool,
        ):
            # resident train shard: 8 chunks (4 hi + 4 lo) of [128, NTOT]
            xn_sb = []
            for k in range(2 * KT):
                t = xn_pool.tile([P, NTOT], fp16, tag=f"xn{k}")
                nc.sync.dma_start(t, xnT.ap()[k * P : (k + 1) * P, :])
                xn_sb.append(t)
            xaug_sb = aug_pool.tile([P, AUGF], fp16)
            nc.sync.dma_start(xaug_sb, xaug.ap())
            ones_sb = ones_pool.tile([P, P], fp16)
            nc.sync.dma_start(ones_sb, ones.ap())

            # (lhsT chunk, rhs chunk): hi*hi, lo*hi, hi*lo
            pairs = (
                [(k, k) for k in range(KT)]
                + [(KT + k, k) for k in range(KT)]
                + [(k, KT + k) for k in range(KT)]
            )

            for m in range(MT):
                xt_sb = xt_pool.tile([P, 2 * KT, P], fp16)
                nc.sync.dma_start(
                    xt_sb,
                    xtT.ap()[:, m * P : (m + 1) * P].rearrange(
                        "(ko p) m -> p ko m", p=P
                    ),
                )
                score_sb = score_pool.tile([P, NTOT], f32)
                for n in range(NTILES):
                    nw = min(512, NTOT - n * 512)
                    prow = 32 * (n % 4)
                    fcol = 512 * (n // 4)
                    ps = psum_pool.tile([P, 512], f32)
                    for i, (tk, nk) in enumerate(pairs):
                        nc.tensor.matmul(
                            ps[:, :nw],
                            xt_sb[:, tk, :],
                            xn_sb[nk][:, n * 512 : n * 512 + nw],
                            start=(i == 0),
                            stop=False,
                        )
                    nc.tensor.matmul(
                        ps[:, :nw],
                        ones_sb[prow : prow + 2, :],
                        xaug_sb[prow : prow + 2, fcol : fcol + nw],
                        start=False,
                        stop=True,
                        tile_position=(prow, 0),
                    )
                    nc.scalar.copy(score_sb[:, n * 512 : n * 512 + nw], ps[:, :nw])
                out_sb = out_pool.tile([P, NCLASSES * 8], f32)
                for c in range(NCLASSES):
                    nc.vector.max(
                        out=out_sb[:, c * 8 : (c + 1) * 8],
                        in_=score_sb[:, c * B : (c + 1) * B],
                    )
                nc.sync.dma_start(topv.ap()[m * P : (m + 1) * P, :], out_sb)
    nc.compile()
    return nc


def _hi_lo(x):
    hi = x.astype(np.float16)
    lo = (x - hi.astype(np.float32)).astype(np.float16)
    return hi, lo


_RUNNER = None


def _get_runner():
    """Build the sharded PJRT callable once (mirrors
    concourse.bass2jax.run_bass_via_pjrt, but cached so repeat calls do not
    re-trace/re-jit, which also enables steady-state timing)."""
    global _RUNNER
    if _RUNNER is not None:
        return _RUNNER
    import jax
    from jax.experimental.shard_map import shard_map
    from jax.sharding import Mesh, PartitionSpec

    import concourse.mybir as mybir
    from concourse.bass2jax import (
        _bass_exec_p,
        install_neuronx_cc_hook,
        partition_id_tensor,
    )

    nc = _build()
    install_neuronx_cc_hook()
    partition_name = nc.partition_id_tensor.name if nc.partition_id_tensor else None

    in_names: list[str] = []
    out_names: list[str] = []
    out_avals = []
    for alloc in nc.m.functions[0].allocations:
        if not isinstance(alloc, mybir.MemoryLocationSet):
            continue
        name = alloc.memorylocations[0].name
        if alloc.kind == "ExternalInput":
            if name != partition_name:
                in_names.append(name)
        elif alloc.kind == "ExternalOutput":
            out_avals.append(
                jax.core.ShapedArray(
                    tuple(alloc.tensor_shape), mybir.dt.np(alloc.dtype)
                )
            )
            out_names.append(name)
    n_params = len(in_names)
    param_names = list(in_names)
    in_names = in_names + out_names
    if partition_name is not None:
        in_names.append(partition_name)
    donate = tuple(range(n_params, n_params + len(out_names)))

    def _body(*args):
        operands = list(args)
        if partition_name is not None:
            operands.append(partition_id_tensor())
        outs = _bass_exec_p.bind(
            *operands,
            out_avals=tuple(out_avals),
            in_names=tuple(in_names),
            out_names=tuple(out_names),
            lowering_input_output_aliases=(),
            sim_require_finite=True,
            sim_require_nnan=True,
            nc=nc,
        )
        return tuple(outs)

    devices = jax.devices()[:NCORES]
    mesh = Mesh(np.asarray(devices), ("core",))
    in_specs = (PartitionSpec("core"),) * (n_params + len(out_names))
    out_specs = (PartitionSpec("core"),) * len(out_names)
    sharded = jax.jit(
        shard_map(
            _body, mesh=mesh, in_specs=in_specs, out_specs=out_specs, check_rep=False
        ),
        donate_argnums=donate,
        keep_unused=True,
    )
    _RUNNER = (sharded, param_names, out_names, out_avals, mesh)
    return _RUNNER


def _execute(in_maps, n_time_runs=0):
    """Run the SPMD kernel; returns per-core dict of outputs.  When
    n_time_runs > 0, also re-runs with on-device inputs and records the
    best wall-clock execution time in LAST_EXEC_TIME_NS."""
    global LAST_EXEC_TIME_NS
    import time as _time

    import jax
    from jax.sharding import NamedSharding, PartitionSpec

    sharded, param_names, out_names, out_avals, mesh = _get_runner()
    concat_in = [
        np.concatenate([np.asarray(m[name]) for m in in_maps], axis=0)
        for name in param_names
    ]

    def _zeros():
        return [
            np.zeros((NCORES * a.shape[0], *a.shape[1:]), a.dtype) for a in out_avals
        ]

    out_arrs = sharded(*concat_in, *_zeros())
    jax.block_until_ready(out_arrs)

    if n_time_runs:
        sh = NamedSharding(mesh, PartitionSpec("core"))
        dev_in = [jax.device_put(x, sh) for x in concat_in]
        jax.block_until_ready(dev_in)
        best = None
        for _ in range(n_time_runs):
            zs = [jax.device_put(z, sh) for z in _zeros()]
            jax.block_until_ready(zs)
            t0 = _time.perf_counter()
            o = sharded(*dev_in, *zs)
            jax.block_until_ready(o)
            dt = _time.perf_counter() - t0
            best = dt if best is None else min(best, dt)
        LAST_EXEC_TIME_NS = int(best * 1e9)

    return [
        {
            name: np.asarray(out_arrs[i]).reshape(NCORES, *out_avals[i].shape)[c]
            for i, name in enumerate(out_names)
        }
        for c in range(NCORES)
    ]


def kernel(X_train, X_test, y_train):
    global LAST_EXEC_TIME_NS

    Xtr = np.ascontiguousarray(np.asarray(X_train, dtype=np.float32))
    Xte = np.ascontiguousarray(np.asarray(X_test, dtype=np.float32))
    y = np.asarray(y_train)
    assert Xtr.shape == (NTRAIN, DIM) and Xte.shape == (NTEST, DIM)

    # ---- host: class-bucketed shard assignment ----
    order = np.argsort(y, kind="stable")
    y_sorted = y[order]
    starts = np.searchsorted(y_sorted, np.arange(NCLASSES + 1))
    core_x = np.zeros((NCORES, NTOT, DIM), np.float32)
    core_real = np.zeros((NCORES, NTOT), bool)
    for c in range(NCLASSES):
        members = order[starts[c] : starts[c + 1]]
        parts = np.array_split(members, NCORES)
        for i in range(NCORES):
            k = len(parts[i])
            assert k <= B, f"bucket overflow: class {c} core {i} has {k} > {B}"
            core_x[i, c * B : c * B + k] = Xtr[parts[i]]
            core_real[i, c * B : c * B + k] = True

    # aug term: -0.5*||x||^2 for real slots, PAD_SCORE for padding
    x2 = -0.5 * np.einsum("cnd,cnd->cn", core_x, core_x, optimize=True)
    x2 = np.where(core_real, x2, np.float32(PAD_SCORE)).astype(np.float32)

    # ---- fp16 hi/lo packing ----
    t_hi, t_lo = _hi_lo(Xte)
    xtT16 = np.ascontiguousarray(
        np.concatenate([t_hi.T, t_lo.T], axis=0)
    )  # [1024, 4096]
    ones16 = np.ones((P, P), np.float16)

    in_maps = []
    for i in range(NCORES):
        n_hi, n_lo = _hi_lo(core_x[i])
        xnT16 = np.ascontiguousarray(np.concatenate([n_hi.T, n_lo.T], axis=0))
        a_hi, a_lo = _hi_lo(x2[i])
        aug_flat_hi = np.full(NTILES * 512, np.float16(PAD_SCORE), np.float16)
        aug_flat_lo = np.zeros(NTILES * 512, np.float16)
        aug_flat_hi[:NTOT] = a_hi
        aug_flat_lo[:NTOT] = a_lo
        xaug = np.zeros((P, AUGF), np.float16)
        for n in range(NTILES):
            f = 512 * (n // 4)
            pr = 32 * (n % 4)
            xaug[pr, f : f + 512] = aug_flat_hi[n * 512 : (n + 1) * 512]
            xaug[pr + 1, f : f + 512] = aug_flat_lo[n * 512 : (n + 1) * 512]
        in_maps.append(
            {"xtT": xtT16, "xnT": xnT16, "xaug": xaug, "ones": ones16}
        )

    # ---- run on 8 cores ----
    n_time_runs = 3 if os.environ.get("KNN_TRACE") else 0
    results = _execute(in_maps, n_time_runs=n_time_runs)

    # ---- host: merge candidates -> top-5 -> mode ----
    vals = np.stack([results[i]["topv"] for i in range(NCORES)])  # [8, 4096, 80]
    # [4096, class, core*8]
    cands = (
        vals.reshape(NCORES, NTEST, NCLASSES, 8)
        .transpose(1, 2, 0, 3)
        .reshape(NTEST, NCLASSES * NCORES * 8)
    )
    labels = np.repeat(np.arange(NCLASSES), NCORES * 8)
    idx5 = np.argpartition(-cands, NNEIGH, axis=1)[:, :NNEIGH]
    nearest = labels[idx5]  # [4096, 5]

    counts = (nearest[:, :, None] == nearest[:, None, :]).sum(-1)
    maxc = counts.max(axis=1, keepdims=True)
    big = np.iinfo(y.dtype).max if np.issubdtype(y.dtype, np.integer) else NCLASSES
    cand_lab = np.where(counts == maxc, nearest, big)
    return cand_lab.min(axis=1).astype(y.dtype)
